# revision 17
# baseline (speedup 1.0000x reference)
"""Trainium2 Bass kernel for nn_ChannelProcessing.

Module: channel attention (softmax over N, sigmoid gate) * value path
(fc1 -> dwconv3x3 -> gelu -> BN -> dwconv3x3 -> gamma*. + residual -> fc2 -> LN).

Sharding: pure data parallel over batch B=32 across 8 cores (4 batches/core).

Per-core layout strategy:
  - x is loaded [N,C] and PE-transposed to xT [C_part, N_free].
  - q GEMM / fc1 / convs run channel-on-partition.
  - Depthwise 3x3 convs run on the TensorEngine as 9 diagonal-matmul taps
    (host-precomputed 32x32 diagonal blocks, 4 concurrent tile_positions)
    accumulating in PSUM, reading a zero-padded [128,30,30] buffer with
    shifted-window access patterns.
  - fc2 uses u2 [HID,N] tiles as lhsT so the output lands [N_part, C_free]:
    LayerNorm reduces along free dim and the result DMAs out contiguously.
  - fp32r (full-rate) matmuls for heavy GEMMs; exact fp32 for transposes
    and rank-1 broadcast/bias matmuls.
"""

import sys

sys.path.insert(0, "/opt/trn_rl_repo")

from contextlib import ExitStack

import ml_dtypes
import numpy as np

import concourse.bass as bass
import concourse.tile as tile
from concourse import bacc, mybir

# ---- problem constants (hardcoded per contract) ----
B, HH, WW, C, HEADS, HID = 32, 28, 28, 384, 8, 1536
N = HH * WW  # 784
NCORES = 8
BP = B // NCORES  # 4 batches per core
D = C // HEADS  # 48
EPS_BN = 1e-5
EPS_LN = 1e-5
GC = C // 128  # 3
GH = HID // 128  # 12
F32 = mybir.dt.float32
F32R = mybir.dt.float32r
BF16 = mybir.dt.bfloat16
AF = mybir.ActivationFunctionType
OP = mybir.AluOpType

# n-chunks of 128 for transpose / fc2 (784 = 6*128 + 16)
NCHUNKS = [(i * 128, min(128, N - i * 128)) for i in range((N + 127) // 128)]
# free-dim halves for 784-column passes (rows 0..13 and 14..27 of the 28x28 grid)
HALVES = [(0, 0, 392), (14, 392, 392)]  # (h0, col0, ncols)

USE_F32R = True
CONV_TILE4 = True  # 4 concurrent 32x32 diagonal tile_positions per conv matmul


def _R(ap):
    return ap


def build_nc(bp=BP, reps=1):
    nc = bacc.Bacc("TRN2", target_bir_lowering=False, debug=False)

    x_d = nc.dram_tensor("x_sh", [bp, N, C], F32, kind="ExternalInput")
    wq_d = nc.dram_tensor("wq_l", [128, GC, C], F32R, kind="ExternalInput")
    fc1w_d = nc.dram_tensor("fc1w_l", [128, GC, HID], F32R, kind="ExternalInput")
    fc2w_d = nc.dram_tensor("fc2w_l", [128, GH, C], F32R, kind="ExternalInput")
    d1_d = nc.dram_tensor("d1_l", [128, GH, 9, 32], BF16, kind="ExternalInput")
    d2_d = nc.dram_tensor("d2_l", [128, GH, 9, 32], BF16, kind="ExternalInput")
    hv_d = nc.dram_tensor("hv_l", [128, 5, GH], F32, kind="ExternalInput")
    cv_d = nc.dram_tensor("cv_l", [1, 3, C], F32, kind="ExternalInput")
    tc_d = nc.dram_tensor("tempc_l", [128, GC], F32, kind="ExternalInput")
    m1_d = nc.dram_tensor("m1_l", [128, GC, HEADS], F32, kind="ExternalInput")
    m2_d = nc.dram_tensor("m2_l", [HEADS, C], F32R, kind="ExternalInput")
    id_d = nc.dram_tensor("id_l", [128, 128], F32, kind="ExternalInput")
    z_d = nc.dram_tensor("z_l", [128, 30, 30], BF16, kind="ExternalInput")
    out_d = nc.dram_tensor("out_sh", [bp, N, C], F32, kind="ExternalOutput")

    with tile.TileContext(nc) as tc:
        with ExitStack() as ctx:
            const = ctx.enter_context(tc.tile_pool(name="const", bufs=1))
            xa_p = ctx.enter_context(tc.tile_pool(name="xa_p", bufs=2))
            xt_p = ctx.enter_context(tc.tile_pool(name="xt_p", bufs=2))
            qx_p = ctx.enter_context(tc.tile_pool(name="qx_p", bufs=1))
            u2_p = ctx.enter_context(tc.tile_pool(name="u2_p", bufs=1))
            pad_p = ctx.enter_context(tc.tile_pool(name="pad_p", bufs=2))
            sc_p = ctx.enter_context(tc.tile_pool(name="sc_p", bufs=2))
            col_p = ctx.enter_context(tc.tile_pool(name="col_p", bufs=2))
            sm_p = ctx.enter_context(tc.tile_pool(name="sm_p", bufs=2))
            uf_p = ctx.enter_context(tc.tile_pool(name="uf_p", bufs=8))
            out_p = ctx.enter_context(tc.tile_pool(name="out_p", bufs=2))
            ps_p = ctx.enter_context(tc.tile_pool(name="ps_p", bufs=6, space="PSUM"))

            def ps(pdim, fdim, nm):
                t = ps_p.tile([128, 512], F32, tag="ps", name=nm)
                return t[:pdim, :fdim]

            # ---- load constants ----
            wq_t = const.tile([128, GC, C], F32R, tag="wq", name="wq_t")
            nc.sync.dma_start(wq_t[:], wq_d[:])
            fc1w_t = const.tile([128, GC, HID], F32R, tag="fc1w", name="fc1w_t")
            nc.sync.dma_start(fc1w_t[:], fc1w_d[:])
            fc2w_t = const.tile([128, GH, C], F32R, tag="fc2w", name="fc2w_t")
            nc.sync.dma_start(fc2w_t[:], fc2w_d[:])
            d1_t = const.tile([128, GH, 9, 32], BF16, tag="d1", name="d1_t")
            nc.sync.dma_start(d1_t[:], d1_d[:])
            d2_t = const.tile([128, GH, 9, 32], BF16, tag="d2", name="d2_t")
            nc.sync.dma_start(d2_t[:], d2_d[:])
            hv_t = const.tile([128, 5, GH], F32, tag="hv", name="hv_t")
            nc.sync.dma_start(hv_t[:], hv_d[:])
            cv_t = const.tile([1, 3, C], F32, tag="cv", name="cv_t")
            nc.sync.dma_start(cv_t[:], cv_d[:])
            tc_t = const.tile([128, GC], F32, tag="tc", name="tc_t")
            nc.sync.dma_start(tc_t[:], tc_d[:])
            m1_t = const.tile([128, GC, HEADS], F32, tag="m1", name="m1_t")
            nc.sync.dma_start(m1_t[:], m1_d[:])
            m2_t = const.tile([HEADS, C], F32R, tag="m2", name="m2_t")
            nc.sync.dma_start(m2_t[:], m2_d[:])
            id_t = const.tile([128, 128], F32, tag="id", name="id_t")
            nc.sync.dma_start(id_t[:], id_d[:])
            ones_t = const.tile([1, 128], F32, tag="ones", name="ones_t")
            nc.vector.memset(ones_t[:], 1.0)
            eps_t = const.tile([128, 1], F32, tag="eps", name="eps_t")
            nc.vector.memset(eps_t[:], EPS_LN)

            # persistent zero-padded conv buffers (ping-pong pairs); the
            # borders stay zero -- all later writers touch only the interior
            pad1s, pad2s = [], []
            for i in range(2):
                t1 = pad_p.tile(
                    [128, 30, 30], BF16, tag=f"pad1_{i}", name=f"pad1_{i}", bufs=1
                )
                nc.sync.dma_start(t1[:], z_d[:])
                pad1s.append(t1)
                t2 = pad_p.tile(
                    [128, 30, 30], BF16, tag=f"pad2_{i}", name=f"pad2_{i}", bufs=1
                )
                nc.sync.dma_start(t2[:], z_d[:])
                pad2s.append(t2)

            for b in [bb for _ in range(reps) for bb in range(bp)]:
                # ---- phase 1: load x[b] and PE-transpose to xT [C,N] ----
                xt = xt_p.tile([128, GC, N], F32R, tag="xt", name="xt")
                for n0, nsz in NCHUNKS:
                    xa = xa_p.tile([128, C], F32, tag="xa", name="xa")
                    nc.sync.dma_start(xa[:nsz, :], x_d[b, n0 : n0 + nsz, :])
                    for cc in range(GC):
                        pt = ps(128, nsz, "pt")
                        nc.tensor.transpose(
                            pt,
                            xa[:nsz, cc * 128 : (cc + 1) * 128],
                            id_t[:nsz, :nsz],
                        )
                        nc.scalar.copy(xt[:, cc, n0 : n0 + nsz], pt)

                # ---- phase 2: q = x@Wq, exp along N ----
                q_exp = qx_p.tile([128, GC, N], BF16, tag="q_exp", name="q_exp")
                qsum = col_p.tile([128, GC, 2], F32, tag="qsum", name="qsum")
                for mg in range(GC):
                    for ch, (_, c0, cw) in enumerate(HALVES):
                        pq = ps(128, cw, "pq")
                        for kc in range(GC):
                            nc.tensor.matmul(
                                pq,
                                lhsT=_R(wq_t[:, kc, mg * 128 : (mg + 1) * 128]),
                                rhs=_R(xt[:, kc, c0 : c0 + cw]),
                                start=(kc == 0),
                                stop=(kc == GC - 1),
                            )
                        nc.scalar.activation(
                            q_exp[:, mg, c0 : c0 + cw],
                            pq,
                            AF.Exp,
                            accum_out=qsum[:, mg, ch : ch + 1],
                        )
                qs1 = col_p.tile([128, GC], F32, tag="qs1", name="qs1")
                nc.vector.reduce_sum(qs1[:], qsum[:], axis=mybir.AxisListType.X)
                rq = col_p.tile([128, GC], F32, tag="rq", name="rq")
                nc.vector.reciprocal(rq[:], qs1[:])

                # ---- phase 3: exp(x) for the k path ----
                x_exp = qx_p.tile([128, GC, N], BF16, tag="x_exp", name="x_exp")
                xsum = col_p.tile([128, GC, 2], F32, tag="xsum", name="xsum")
                for cc in range(GC):
                    for ch, (_, c0, cw) in enumerate(HALVES):
                        nc.scalar.activation(
                            x_exp[:, cc, c0 : c0 + cw],
                            xt[:, cc, c0 : c0 + cw],
                            AF.Exp,
                            accum_out=xsum[:, cc, ch : ch + 1],
                        )
                xs1 = col_p.tile([128, GC], F32, tag="xs1", name="xs1")
                nc.vector.reduce_sum(xs1[:], xsum[:], axis=mybir.AxisListType.X)
                rx = col_p.tile([128, GC], F32, tag="rx", name="rx")
                nc.vector.reciprocal(rx[:], xs1[:])

                # ---- phase 4: k_mean [HEADS, N] = mask1^T @ softmax(x) ----
                m1s = col_p.tile([128, GC, HEADS], BF16, tag="m1s", name="m1s")
                for cc in range(GC):
                    nc.vector.tensor_scalar_mul(
                        m1s[:, cc, :], m1_t[:, cc, :], rx[:, cc : cc + 1]
                    )
                km = sm_p.tile([HEADS, N], F32R, tag="km", name="km", bufs=1)
                for _, c0, cw in HALVES:
                    pkm = ps(HEADS, cw, "pkm")
                    for cc in range(GC):
                        nc.tensor.matmul(
                            pkm,
                            lhsT=m1s[:, cc, :],
                            rhs=x_exp[:, cc, c0 : c0 + cw],
                            start=(cc == 0),
                            stop=(cc == GC - 1),
                        )
                    nc.scalar.copy(km[:, c0 : c0 + cw], pkm)

                # ---- phase 5: attn_raw[c] = sum_n q_sm[c,n] * k_mean[h(c),n] ----
                aparts = col_p.tile([128, GC, 2], F32, tag="aparts", name="aparts")
                for mg in range(GC):
                    for ch, (_, c0, cw) in enumerate(HALVES):
                        pkb = ps(128, cw, "pkb")
                        nc.tensor.matmul(
                            pkb,
                            lhsT=_R(m2_t[:, mg * 128 : (mg + 1) * 128]),
                            rhs=_R(km[:, c0 : c0 + cw]),
                            start=True,
                            stop=True,
                        )
                        sc = sc_p.tile([128, 392], F32, tag="attn_sc", name="attn_sc", bufs=1)
                        nc.vector.scalar_tensor_tensor(
                            out=sc[:, :cw],
                            in0=q_exp[:, mg, c0 : c0 + cw],
                            scalar=rq[:, mg : mg + 1],
                            in1=pkb,
                            op0=OP.mult,
                            op1=OP.mult,
                            accum_out=aparts[:, mg, ch : ch + 1],
                        )
                araw = col_p.tile([128, GC], F32, tag="araw", name="araw")
                nc.vector.reduce_sum(araw[:], aparts[:], axis=mybir.AxisListType.X)
                asig = col_p.tile([128, GC], F32, tag="asig", name="asig")
                nc.scalar.activation(asig[:], araw[:], AF.Sigmoid)
                acol = col_p.tile([128, GC], F32, tag="acol", name="acol")
                nc.vector.tensor_tensor(acol[:], asig[:], tc_t[:], op=OP.mult)

                # ---- phase 6: attn row, w_eff/b_eff, broadcast to [128, C] ----
                arow = sm_p.tile([1, C], F32, tag="arow", name="arow", bufs=1)
                for mg in range(GC):
                    prow = ps(1, 128, "prow")
                    nc.tensor.transpose(prow, acol[:, mg : mg + 1], id_t[:])
                    nc.scalar.copy(arow[:, mg * 128 : (mg + 1) * 128], prow)
                weff = sm_p.tile([1, C], F32, tag="weff", name="weff", bufs=1)
                nc.vector.tensor_tensor(weff[:], arow[:], cv_t[0:1, 1, :], op=OP.mult)
                beff = sm_p.tile([1, C], F32, tag="beff", name="beff", bufs=1)
                nc.vector.tensor_tensor(beff[:], arow[:], cv_t[0:1, 2, :], op=OP.mult)
                web = sm_p.tile([128, C], F32, tag="web", name="web", bufs=1)
                pwb = ps(128, C, "pwb")
                nc.tensor.matmul(pwb, lhsT=ones_t[:], rhs=weff[:], start=True, stop=True)
                nc.scalar.copy(web[:], pwb)
                beb = sm_p.tile([128, C], F32, tag="beb", name="beb", bufs=1)
                pbb = ps(128, C, "pbb")
                nc.tensor.matmul(pbb, lhsT=ones_t[:], rhs=beff[:], start=True, stop=True)
                nc.scalar.copy(beb[:], pbb)
                # fc2 bias broadcast [128, C] (added during fc2 psum evacuation)
                fb = sm_p.tile([128, C], F32, tag="fb", name="fb", bufs=1)
                pfb = ps(128, C, "pfb")
                nc.tensor.matmul(
                    pfb, lhsT=ones_t[:], rhs=cv_t[0:1, 0, :], start=True, stop=True
                )
                nc.scalar.copy(fb[:], pfb)

                # ---- phase 7: MLP value path, per 128-channel hid group ----
                u2 = u2_p.tile([128, GH, N], F32R, tag="u2", name="u2")
                for g in range(GH):
                    pad1 = pad1s[(b * GH + g) % 2]

                    # fc1 for this group -> pad1 interior (bias fused in evac)
                    for h0, c0, cw in HALVES:
                        pu = ps(128, cw, "pu")
                        for kc in range(GC):
                            nc.tensor.matmul(
                                pu,
                                lhsT=_R(fc1w_t[:, kc, g * 128 : (g + 1) * 128]),
                                rhs=_R(xt[:, kc, c0 : c0 + cw]),
                                start=(kc == 0),
                                stop=(kc == GC - 1),
                            )
                        nc.scalar.activation(
                            pad1[:, 1 + h0 : 15 + h0, 1:29],
                            pu,
                            AF.Identity,
                            bias=hv_t[:, 0, g : g + 1],
                        )

                    # conv1 -> gelu -> BN -> pad2 interior
                    pad2 = pad2s[(b * GH + g) % 2]
                    for h0, c0, cw in HALVES:
                        pc = ps(128, cw, "pc")
                        for tap in range(9):
                            dy, dx = tap // 3, tap % 3
                            rhs_w = pad1[:, h0 + dy : h0 + dy + 14, dx : dx + 28]
                            if CONV_TILE4:
                                for t4 in range(4):
                                    s = slice(32 * t4, 32 * t4 + 32)
                                    nc.tensor.matmul(
                                        pc[s, :],
                                        lhsT=_R(d1_t[s, g, tap, :]),
                                        rhs=_R(rhs_w[s, :, :]),
                                        start=(tap == 0),
                                        stop=(tap == 8),
                                        tile_position=(32 * t4, 32 * t4),
                                        skip_group_check=True,
                                    )
                            else:
                                nc.tensor.matmul(
                                    pc,
                                    lhsT=_R(d1_t[:, g, tap, :]),
                                    rhs=_R(rhs_w),
                                    start=(tap == 0),
                                    stop=(tap == 8),
                                )
                        gsc = sc_p.tile([128, 392], F32, tag="gsc", name="gsc")
                        nc.scalar.activation(
                            gsc[:, :cw], pc, AF.Gelu, bias=hv_t[:, 1, g : g + 1]
                        )
                        nc.vector.tensor_scalar(
                            out=pad2[:, 1 + h0 : 15 + h0, 1:29],
                            in0=gsc[:, :cw],
                            scalar1=hv_t[:, 2, g : g + 1],
                            scalar2=hv_t[:, 3, g : g + 1],
                            op0=OP.mult,
                            op1=OP.add,
                        )

                    # conv2 -> u2 = (conv2 + gamma*b2) + u (residual from pad1)
                    for h0, c0, cw in HALVES:
                        pc2 = ps(128, cw, "pc2")
                        for tap in range(9):
                            dy, dx = tap // 3, tap % 3
                            rhs_w = pad2[:, h0 + dy : h0 + dy + 14, dx : dx + 28]
                            if CONV_TILE4:
                                for t4 in range(4):
                                    s = slice(32 * t4, 32 * t4 + 32)
                                    nc.tensor.matmul(
                                        pc2[s, :],
                                        lhsT=_R(d2_t[s, g, tap, :]),
                                        rhs=_R(rhs_w[s, :, :]),
                                        start=(tap == 0),
                                        stop=(tap == 8),
                                        tile_position=(32 * t4, 32 * t4),
                                        skip_group_check=True,
                                    )
                            else:
                                nc.tensor.matmul(
                                    pc2,
                                    lhsT=_R(d2_t[:, g, tap, :]),
                                    rhs=_R(rhs_w),
                                    start=(tap == 0),
                                    stop=(tap == 8),
                                )
                        nc.vector.scalar_tensor_tensor(
                            out=u2[:, g, c0 : c0 + cw],
                            in0=pc2,
                            scalar=hv_t[:, 4, g : g + 1],
                            in1=pad1[:, 1 + h0 : 15 + h0, 1:29],
                            op0=OP.add,
                            op1=OP.add,
                        )

                # ---- phase 8: fc2 -> [N_part, C_free], LayerNorm, attn scale ----
                usum = col_p.tile([128, len(NCHUNKS)], F32, tag="usum", name="usum")
                usq = col_p.tile([128, len(NCHUNKS)], F32, tag="usq", name="usq")
                nc.vector.memset(usum[:], 0.0)
                nc.vector.memset(usq[:], 1.0)
                ufs = []
                for ci, (n0, nsz) in enumerate(NCHUNKS):
                    pf = ps(nsz, C, "pf")
                    for kg in range(GH):
                        nc.tensor.matmul(
                            pf,
                            lhsT=_R(u2[:, kg, n0 : n0 + nsz]),
                            rhs=_R(fc2w_t[:, kg, :]),
                            start=(kg == 0),
                            stop=(kg == GH - 1),
                        )
                    uf = uf_p.tile([128, C], F32, tag="uf", name="uf")
                    nc.vector.scalar_tensor_tensor(
                        out=uf[:nsz, :],
                        in0=pf,
                        scalar=0.0,
                        in1=fb[:nsz, :],
                        op0=OP.add,
                        op1=OP.add,
                        accum_out=usum[:nsz, ci : ci + 1],
                    )
                    sqs = sc_p.tile([128, C], F32, tag="sqs", name="sqs", bufs=1)
                    nc.scalar.activation(
                        sqs[:nsz, :], uf[:nsz, :], AF.Square,
                        accum_out=usq[:nsz, ci : ci + 1],
                    )
                    ufs.append(uf)

                nch = len(NCHUNKS)
                mu = col_p.tile([128, nch], F32, tag="mu", name="mu")
                nc.vector.tensor_scalar_mul(mu[:], usum[:], 1.0 / C)
                ex2 = col_p.tile([128, nch], F32, tag="ex2", name="ex2")
                nc.vector.tensor_scalar_mul(ex2[:], usq[:], 1.0 / C)
                musq = col_p.tile([128, nch], F32, tag="musq", name="musq")
                nc.vector.tensor_tensor(musq[:], mu[:], mu[:], op=OP.mult)
                var = col_p.tile([128, nch], F32, tag="var", name="var")
                nc.vector.tensor_sub(var[:], ex2[:], musq[:])
                std = col_p.tile([128, nch], F32, tag="std", name="std")
                nc.scalar.activation(std[:], var[:], AF.Sqrt, bias=eps_t[:, 0:1])
                istd = col_p.tile([128, nch], F32, tag="istd", name="istd")
                nc.vector.reciprocal(istd[:], std[:])

                for ci, (n0, nsz) in enumerate(NCHUNKS):
                    uf = ufs[ci]
                    t1 = sc_p.tile([128, C], F32, tag="t1", name="t1")
                    nc.vector.scalar_tensor_tensor(
                        out=t1[:nsz, :],
                        in0=uf[:nsz, :],
                        scalar=mu[:nsz, ci : ci + 1],
                        in1=web[:nsz, :],
                        op0=OP.subtract,
                        op1=OP.mult,
                    )
                    ot = out_p.tile([128, C], F32, tag="ot", name="ot")
                    nc.vector.scalar_tensor_tensor(
                        out=ot[:nsz, :],
                        in0=t1[:nsz, :],
                        scalar=istd[:nsz, ci : ci + 1],
                        in1=beb[:nsz, :],
                        op0=OP.mult,
                        op1=OP.add,
                    )
                    nc.sync.dma_start(out_d[b, n0 : n0 + nsz, :], ot[:nsz, :])

    nc.compile()
    return nc


def make_host_inputs(
    Wq, temperature, fc1_w, fc1_b, gamma, conv1_w, conv1_b,
    bn_w, bn_b, bn_mean, bn_var, conv2_w, conv2_b, fc2_w, fc2_b, ln_w, ln_b,
):
    f = lambda a: np.ascontiguousarray(np.asarray(a, dtype=np.float32))
    Wq, fc1_w, fc2_w = f(Wq), f(fc1_w), f(fc2_w)
    fc1_b, gamma, conv1_b = f(fc1_b), f(gamma), f(conv1_b)
    bn_w, bn_b, bn_mean, bn_var = f(bn_w), f(bn_b), f(bn_mean), f(bn_var)
    conv2_b, fc2_b, ln_w, ln_b = f(conv2_b), f(fc2_b), f(ln_w), f(ln_b)
    temperature = f(temperature).reshape(HEADS)

    s_bn = bn_w / np.sqrt(bn_var + EPS_BN)
    b_bn = bn_b - bn_mean * s_bn
    w1t = f(conv1_w).reshape(HID, 9)
    w2t = f(conv2_w).reshape(HID, 9) * gamma[:, None]
    gb2 = gamma * conv2_b

    def diag_blocks(wt):  # wt [HID, 9] -> [128, GH, 9, 32]
        r = wt.reshape(GH, 128, 9)  # [g, p, t]
        d = np.zeros((128, GH, 9, 32), np.float32)
        for p in range(128):
            d[p, :, :, p % 32] = r[:, p, :]
        return d

    hv = np.stack([fc1_b, conv1_b, s_bn, b_bn, gb2])  # [5, HID]
    cv = np.stack([fc2_b, ln_w, ln_b])[None]  # [1, 3, C]
    tempc = np.repeat(temperature, D)  # [C]
    m1 = np.zeros((C, HEADS), np.float32)
    m1[np.arange(C), np.arange(C) // D] = 1.0 / D
    m2 = np.zeros((HEADS, C), np.float32)
    m2[np.arange(C) // D, np.arange(C)] = 1.0

    return {
        "wq_l": np.ascontiguousarray(Wq.reshape(GC, 128, C).transpose(1, 0, 2)),
        "fc1w_l": np.ascontiguousarray(fc1_w.reshape(GC, 128, HID).transpose(1, 0, 2)),
        "fc2w_l": np.ascontiguousarray(fc2_w.reshape(GH, 128, C).transpose(1, 0, 2)),
        "d1_l": diag_blocks(w1t).astype(ml_dtypes.bfloat16),
        "d2_l": diag_blocks(w2t).astype(ml_dtypes.bfloat16),
        "hv_l": np.ascontiguousarray(hv.reshape(5, GH, 128).transpose(2, 0, 1)),
        "cv_l": np.ascontiguousarray(cv),
        "tempc_l": np.ascontiguousarray(tempc.reshape(GC, 128).T),
        "m1_l": np.ascontiguousarray(m1.reshape(GC, 128, HEADS).transpose(1, 0, 2)),
        "m2_l": np.ascontiguousarray(m2),
        "id_l": np.eye(128, dtype=np.float32),
        "z_l": np.zeros((128, 30, 30), ml_dtypes.bfloat16),
    }


_NC_CACHE = {}


def kernel(
    x, Wq, temperature, fc1_w, fc1_b, gamma, conv1_w, conv1_b,
    bn_w, bn_b, bn_mean, bn_var, conv2_w, conv2_b, fc2_w, fc2_b,
    ln_w, ln_b, H, W, **_unused,
):
    from concourse.bass_utils import run_bass_kernel_spmd

    assert int(H) == HH and int(W) == WW

    x = np.ascontiguousarray(np.asarray(x, dtype=np.float32))
    assert x.shape == (B, N, C)

    if "nc" not in _NC_CACHE:
        _NC_CACHE["nc"] = build_nc()
    nc = _NC_CACHE["nc"]

    common = make_host_inputs(
        Wq, temperature, fc1_w, fc1_b, gamma, conv1_w, conv1_b,
        bn_w, bn_b, bn_mean, bn_var, conv2_w, conv2_b, fc2_w, fc2_b, ln_w, ln_b,
    )
    in_maps = [
        {**common, "x_sh": np.ascontiguousarray(x[i * BP : (i + 1) * BP])}
        for i in range(NCORES)
    ]
    res = run_bass_kernel_spmd(nc, in_maps, core_ids=list(range(NCORES)))
    out = np.concatenate([r["out_sh"] for r in res.results], axis=0)
    y2 = np.ascontiguousarray(out.reshape(B, N, HEADS, D).transpose(0, 2, 1, 3))
    return out, y2
